# revision 1
# baseline (speedup 1.0000x reference)
"""Trainium2 Bass kernel for CustomMamba (d_model=64, d_inner=128, d_state=16,
d_conv=4, dt_rank=4) over x:(4,128,64,64).

Sharding: data-parallel over the (b*n)=256 effective-batch axis, 32 sequences
per core across 8 cores; small Mamba params replicated.

Per-core layout: d_inner=128 on SBUF partitions. The selective scan runs as a
DVE tensor_tensor_scan per sequence-group along a flattened (state, seq, t)
free dim; segment resets are done by forcing the decay dA=exp(dt*A) to 0 at
t=0 of every segment (exponent preset to -1e30). B/C (state-indexed) are
replicated across the d-partitions via a DRAM round-trip broadcast DMA, so
the b-build and C-multiply are big bf16 tensor_tensor ops, split between the
Vector and GPSIMD engines to balance load.

Hardware quirks handled:
- only the exp/ln activation table set is used (softplus = ln(1+exp(x)),
  sigmoid = exp(-ln(1+exp(-x)))) so no table switches occur;
- instructions can carry one sync wait in this toolchain, so constants + x
  arrive in a single packed DMA and a post-scheduling pass splits remaining
  multi-wait instructions into chained drains.
"""

import numpy as np

B, T, N, F = 4, 128, 64, 64          # x shape (b, t, n, f)
D = 128                               # d_inner
K = 16                                # d_state
R = 4                                 # dt_rank
DC = 4                                # d_conv
NCORES = 8
SEQ = (B * N) // NCORES               # 32 sequences per core
GS = 4                                # sequences per group
NG = SEQ // GS                        # 8 groups
GCOLS = GS * T                        # 512 free columns per group
TPAD = T + DC - 1                     # 131 padded time

# engine-split knobs
EB_ACT = 6                            # e-build planes on ScalarE (rest DVE)
PB_GPS = 3                            # b-build planes on GPSIMD (rest DVE)
PC_GPS = 3                            # C-mul planes on GPSIMD
PT_GPS = 1                            # tree-L1 adds on GPSIMD

# packed-constant column offsets (f32 words per partition)
C_WFOLD = 0                 # [64, DC*D]      512
C_WZ = C_WFOLD + DC * D     # [64, D]         128
C_WXBC = C_WZ + D           # [128, 2K]       32
C_WDTX = C_WXBC + 2 * K     # [128, D]        128
C_WOUT = C_WDTX + D         # [128, F]        64
C_XPAD = C_WOUT + F         # [64, SEQ*TPAD]  4192
C_BDT = C_XPAD + SEQ * TPAD
C_CVB = C_BDT + 1
C_NCVB = C_CVB + 1
C_DP = C_NCVB + 1
C_A = C_DP + 1              # [128, K]
PACK_COLS = C_A + K

_CACHE = {}


def _build_program():
    import concourse.bass as bass
    import concourse.mybir as mybir
    import concourse.tile as tile

    fp32 = mybir.dt.float32
    bf16 = mybir.dt.bfloat16
    AL = mybir.AluOpType
    AF = mybir.ActivationFunctionType

    nc = bass.Bass(
        "TRN2",
        target_bir_lowering=False,
        debug=False,
        enable_asserts=False,
        num_devices=NCORES,
    )

    d_pack = nc.dram_tensor("pack", [D, PACK_COLS], fp32, kind="ExternalInput")
    d_out = nc.dram_tensor("yout", [T, SEQ, F], fp32, kind="ExternalOutput")

    with tile.TileContext(nc) as tc:
        with (
            tc.tile_pool(name="const", bufs=1) as cpool,
            tc.tile_pool(name="ap2", bufs=2) as ap2,
            tc.tile_pool(name="ap1", bufs=1) as ap1,
            tc.tile_pool(name="tmp4", bufs=4) as tmp4,
            tc.tile_pool(name="spE", bufs=2) as spE,
            tc.tile_pool(name="spB", bufs=2) as spB,
            tc.tile_pool(name="spH", bufs=2) as spH,
            tc.tile_pool(name="spT", bufs=1) as spT,
            tc.tile_pool(name="bc", bufs=2) as bcp,
            tc.tile_pool(name="dram", bufs=2, space="DRAM") as dpool,
            tc.tile_pool(name="psA", bufs=1, space="PSUM") as psA,
            tc.tile_pool(name="psX", bufs=2, space="PSUM") as psX,
            tc.tile_pool(name="psO", bufs=3, space="PSUM") as psO,
        ):
            wpk = cpool.tile([D, PACK_COLS], fp32)
            nc.sync.dma_start(wpk[:], d_pack[:])

            wfoldA = wpk[:, C_WFOLD : C_WFOLD + D]
            wfoldB = wpk[:, C_WFOLD + D : C_WFOLD + 2 * D]
            wz = wpk[0:F, C_WZ : C_WZ + D]
            wxBC = wpk[:, C_WXBC : C_WXBC + 2 * K]
            wdtx = wpk[:, C_WDTX : C_WDTX + D]
            wout = wpk[:, C_WOUT : C_WOUT + F]
            xpad = wpk[:, C_XPAD : C_XPAD + SEQ * TPAD].rearrange(
                "p (n t) -> p n t", n=SEQ
            )
            bdt = wpk[:, C_BDT : C_BDT + 1]
            cvb = wpk[:, C_CVB : C_CVB + 1]
            ncvb = wpk[:, C_NCVB : C_NCVB + 1]
            dp = wpk[:, C_DP : C_DP + 1]
            At = wpk[:, C_A : C_A + K]

            import contextlib

            for g in range(NG):
                q0 = g * GS
                # raise scheduling priority of the group front so it
                # overlaps the previous group's tail
                _prio = (
                    tc.high_priority(offset=80) if g > 0 else contextlib.nullcontext()
                )
                _prio.__enter__()
                # -- u_lin = causal_conv(x @ WuT) (conv folded into 4 taps)
                u_ps = psA.tile([D, GCOLS], fp32, tag="ups")
                nc.tensor.matmul(
                    u_ps[:], wfoldA[:], xpad[:, q0 : q0 + GS, 0:T],
                    start=True, stop=False,
                )
                nc.tensor.matmul(
                    u_ps[:], wfoldB[:], xpad[:, q0 : q0 + GS, 2 : 2 + T],
                    start=False, stop=True,
                )
                # silu(x+cb) = (x+cb) * exp(-ln(1+exp(-x-cb)))
                ta = tmp4.tile([D, GCOLS], fp32, tag="tmp")
                nc.scalar.activation(ta[:], u_ps[:], AF.Exp, bias=ncvb, scale=-1.0)
                tb = tmp4.tile([D, GCOLS], fp32, tag="tmp")
                nc.scalar.activation(tb[:], ta[:], AF.Ln, bias=1.0)
                tsg = tmp4.tile([D, GCOLS], fp32, tag="tmp")
                nc.scalar.activation(tsg[:], tb[:], AF.Exp, scale=-1.0)
                u_c = ap2.tile([D, GCOLS], fp32, tag="u_c")
                nc.vector.scalar_tensor_tensor(
                    u_c[:], u_ps[:], cvb, tsg[:], op0=AL.add, op1=AL.mult
                )

                # -- z path: sz = silu(z)
                z_ps = psA.tile([D, GCOLS], fp32, tag="zps")
                nc.tensor.matmul(
                    z_ps[:], wz[:], xpad[0:F, q0 : q0 + GS, DC - 1 : TPAD],
                    start=True, stop=True,
                )
                za = tmp4.tile([D, GCOLS], fp32, tag="tmp")
                nc.scalar.activation(za[:], z_ps[:], AF.Exp, scale=-1.0)
                zb = tmp4.tile([D, GCOLS], fp32, tag="tmp")
                nc.scalar.activation(zb[:], za[:], AF.Ln, bias=1.0)
                zsg = tmp4.tile([D, GCOLS], fp32, tag="tmp")
                nc.scalar.activation(zsg[:], zb[:], AF.Exp, scale=-1.0)
                sz = ap2.tile([D, GCOLS], fp32, tag="sz")
                nc.vector.tensor_mul(sz[:], z_ps[:], zsg[:])

                # -- B, C rows of x_dbl -> bf16 (ACT evac) -> DRAM -> partition-broadcast
                bt_ps = psX.tile([K, GCOLS], fp32, tag="xdps")
                nc.tensor.matmul(bt_ps[:], wxBC[:, 0:K], u_c[:], start=True, stop=True)
                Btb = ap2.tile([K, GCOLS], bf16, tag="Btb")
                nc.scalar.copy(Btb[:], bt_ps[:])
                drB = dpool.tile([K, GCOLS], bf16, tag="drB")
                nc.sync.dma_start(drB[:], Btb[:])
                Bb = bcp.tile([D, K, GCOLS], bf16, tag="bc")
                nc.sync.dma_start(
                    Bb[:], drB[:].unsqueeze(0).broadcast_to([D, K, GCOLS])
                )

                ct_ps = psX.tile([K, GCOLS], fp32, tag="xdps")
                nc.tensor.matmul(
                    ct_ps[:], wxBC[:, K : 2 * K], u_c[:], start=True, stop=True
                )
                Ctb = ap2.tile([K, GCOLS], bf16, tag="Ctb")
                nc.scalar.copy(Ctb[:], ct_ps[:])
                drC = dpool.tile([K, GCOLS], bf16, tag="drC")
                nc.scalar.dma_start(drC[:], Ctb[:])
                Cb = bcp.tile([D, K, GCOLS], bf16, tag="bc")
                nc.scalar.dma_start(
                    Cb[:], drC[:].unsqueeze(0).broadcast_to([D, K, GCOLS])
                )

                # -- dt = softplus(u_c @ WdtxT + b_dt) = ln(1+exp(lin+b))
                dt_ps = psA.tile([D, GCOLS], fp32, tag="dtps")
                nc.tensor.matmul(dt_ps[:], wdtx[:], u_c[:], start=True, stop=True)
                dta = tmp4.tile([D, GCOLS], fp32, tag="tmp")
                nc.scalar.activation(dta[:], dt_ps[:], AF.Exp, bias=bdt)
                dt = ap2.tile([D, GCOLS], fp32, tag="dt")
                nc.scalar.activation(dt[:], dta[:], AF.Ln, bias=1.0)
                dtb = ap2.tile([D, GCOLS], bf16, tag="dtb")
                nc.vector.tensor_copy(dtb[:], dt[:])

                # -- dtu = dt * u_c (bf16, feeds b-build broadcast multiply)
                dtu = ap2.tile([D, GCOLS], bf16, tag="dtu")
                nc.gpsimd.tensor_mul(dtu[:], dt[:], u_c[:])

                # -- decay exponent e[:, s, q, t] = dt * A[:, s]; -1e30 at t=0
                e = spE.tile([D, K, GS, T], bf16, tag="e")
                for s in range(K):
                    pl = e[:, s, :, :].rearrange("p q t -> p (q t)")
                    if s < EB_ACT:
                        nc.scalar.activation(
                            pl, dt[:], AF.Copy, scale=At[:, s : s + 1]
                        )
                    else:
                        nc.vector.tensor_scalar_mul(pl, dtb[:], At[:, s : s + 1])
                nc.gpsimd.memset(e[:, :, :, 0:1], -1e30)
                eflat = e[:].rearrange("p s q t -> p (s q t)")
                nc.scalar.activation(eflat, eflat, AF.Exp)

                # -- b = dtu (bcast over s) * Bb   [split DVE / GPSIMD]
                bmat = spB.tile([D, K, GS, T], bf16, tag="b")
                bmv = bmat[:].rearrange("p s q t -> p s (q t)")
                dtub = dtu[:, None, :].broadcast_to([D, K, GCOLS])
                nc.vector.tensor_mul(
                    bmv[:, PB_GPS:K, :], dtub[:, PB_GPS:K, :], Bb[:, PB_GPS:K, :]
                )
                if PB_GPS:
                    nc.gpsimd.tensor_mul(
                        bmv[:, 0:PB_GPS, :], dtub[:, 0:PB_GPS, :], Bb[:, 0:PB_GPS, :]
                    )

                _prio.__exit__(None, None, None)
                # -- selective scan: h = dA*h + b along (s, q, t) flat
                h = spH.tile([D, K, GS, T], bf16, tag="h")
                nc.vector.tensor_tensor_scan(
                    h[:].rearrange("p s q t -> p (s q t)"),
                    eflat,
                    bmat[:].rearrange("p s q t -> p (s q t)"),
                    0.0,
                    op0=AL.mult,
                    op1=AL.add,
                )

                # -- ymul = h * Cb (into b slot); tree-reduce over s
                ymul = spB.tile([D, K, GS, T], bf16, tag="b")  # reuse b slot
                ymv = ymul[:].rearrange("p s q t -> p s (q t)")
                hv = h[:].rearrange("p s q t -> p s (q t)")
                nc.vector.tensor_mul(
                    ymv[:, PC_GPS:K, :], hv[:, PC_GPS:K, :], Cb[:, PC_GPS:K, :]
                )
                if PC_GPS:
                    nc.gpsimd.tensor_mul(
                        ymv[:, 0:PC_GPS, :], hv[:, 0:PC_GPS, :], Cb[:, 0:PC_GPS, :]
                    )
                trt = spT.tile([D, 14, GCOLS], bf16, tag="tr")
                tr = trt[:]
                ym4 = ymul[:].rearrange("p (a b) q t -> p a b (q t)", a=8)
                if PT_GPS:
                    nc.gpsimd.tensor_add(
                        tr[:, 0:PT_GPS, :], ym4[:, 0:PT_GPS, 0, :], ym4[:, 0:PT_GPS, 1, :]
                    )
                nc.vector.tensor_add(
                    tr[:, PT_GPS:8, :], ym4[:, PT_GPS:8, 0, :], ym4[:, PT_GPS:8, 1, :]
                )
                tr4 = trt[:].rearrange("p (a b) n -> p a b n", a=7)[:, 0:4]
                nc.vector.tensor_add(tr[:, 8:12, :], tr4[:, :, 0, :], tr4[:, :, 1, :])
                tr2 = trt[:, 8:12, :].rearrange("p (a b) n -> p a b n", a=2)
                nc.vector.tensor_add(tr[:, 12:14, :], tr2[:, :, 0, :], tr2[:, :, 1, :])
                y = tmp4.tile([D, GCOLS], fp32, tag="yy")
                nc.vector.tensor_add(y[:], tr[:, 12, :], tr[:, 13, :])

                # -- y2 = y + u_c * Dp ; y3 = y2 * sz
                y2 = tmp4.tile([D, GCOLS], fp32, tag="yy")
                nc.vector.scalar_tensor_tensor(
                    y2[:], u_c[:], dp, y[:], op0=AL.mult, op1=AL.add
                )
                y3 = tmp4.tile([D, GCOLS], fp32, tag="yy")
                nc.vector.tensor_mul(y3[:], y2[:], sz[:])

                # -- out = y3.T @ WoutT per sequence -> [t, f] -> DRAM
                y3v = y3[:].rearrange("p (q t) -> p q t", q=GS)
                osb = ap2.tile([T, GS, F], fp32, tag="osb")
                for q in range(GS):
                    o_ps = psO.tile([T, F], fp32, tag="ops")
                    nc.tensor.matmul(
                        o_ps[:], y3v[:, q, :], wout[:], start=True, stop=True
                    )
                    nc.scalar.copy(osb[:, q, :], o_ps[:])
                nc.scalar.dma_start(d_out[:, q0 : q0 + GS, :], osb[:])

    _legalize_waits(nc)
    return nc


def _legalize_waits(nc):
    """This walrus build allows one sync wait per instruction struct; split
    multi-wait instructions by inserting per-engine drains that each carry
    one of the extra waits."""
    import concourse.mybir as mybir

    n = 0
    for f in nc.m.functions:
        for b in f.blocks:
            out = []
            for i in list(b.instructions):
                si = i.sync_info
                w = list(si.on_wait) if si else []
                if len(w) > 1:
                    for extra in w[:-1]:
                        d = mybir.InstDrain(name=f"I-lgl{n}", ins=[], outs=[])
                        n += 1
                        d.engine = i.engine
                        d.sync_info = mybir.SyncInfo(on_wait=[extra], on_update=[])
                        out.append(d)
                    i.sync_info = mybir.SyncInfo(
                        on_wait=[w[-1]], on_update=list(si.on_update)
                    )
                out.append(i)
            b.instructions = out


def _prep_pack(inputs):
    """Host-side packing of all constants (tiny tensors only)."""
    W_in = np.asarray(inputs["W_in"], np.float32)
    conv_w = np.asarray(inputs["conv_w"], np.float32)
    conv_b = np.asarray(inputs["conv_b"], np.float32)
    W_x = np.asarray(inputs["W_x"], np.float32)
    W_dt = np.asarray(inputs["W_dt"], np.float32)
    b_dt = np.asarray(inputs["b_dt"], np.float32)
    A_log = np.asarray(inputs["A_log"], np.float32)
    Dp = np.asarray(inputs["Dp"], np.float32)
    W_out = np.asarray(inputs["W_out"], np.float32)

    pack = np.zeros((D, PACK_COLS), np.float32)
    WuT = W_in[0:D, :].T                                  # [F, D]
    wfold = WuT[:, None, :] * conv_w.T[None, :, :]        # [F, DC, D]
    # k=128-folded conv weights: rows (f, k2) pairs for taps (0,1) and (2,3)
    pack[0:F, C_WFOLD : C_WFOLD + D] = wfold[:, 0, :]
    pack[F:D, C_WFOLD : C_WFOLD + D] = wfold[:, 1, :]
    pack[0:F, C_WFOLD + D : C_WFOLD + 2 * D] = wfold[:, 2, :]
    pack[F:D, C_WFOLD + D : C_WFOLD + 2 * D] = wfold[:, 3, :]
    pack[0:F, C_WZ : C_WZ + D] = W_in[D : 2 * D, :].T
    pack[:, C_WXBC : C_WXBC + K] = W_x[R : R + K, :].T
    pack[:, C_WXBC + K : C_WXBC + 2 * K] = W_x[R + K : R + 2 * K, :].T
    pack[:, C_WDTX : C_WDTX + D] = (W_dt @ W_x[0:R, :]).T
    pack[:, C_WOUT : C_WOUT + F] = W_out.T
    pack[:, C_BDT] = b_dt
    pack[:, C_CVB] = conv_b
    pack[:, C_NCVB] = -conv_b
    pack[:, C_DP] = Dp
    pack[:, C_A : C_A + K] = -np.exp(A_log)
    return pack


def kernel(**inputs):
    from concourse.bass_utils import run_bass_kernel_spmd

    if "nc" not in _CACHE:
        _CACHE["nc"] = _build_program()
    nc = _CACHE["nc"]

    x = np.asarray(inputs["x"], np.float32)              # (b, t, n, f)
    base_pack = _prep_pack(inputs)

    in_maps = []
    for c in range(NCORES):
        flat0 = c * SEQ                                   # (b*n) start index
        b0, n0 = divmod(flat0, N)
        pk = base_pack.copy()
        xs = x[b0, :, n0 : n0 + SEQ, :].transpose(2, 1, 0)     # [f, n, t]
        xp = pk[:, C_XPAD : C_XPAD + SEQ * TPAD].reshape(D, SEQ, TPAD)
        xp[0:F, :, DC - 1 :] = xs
        xp[F:D, :, 0 : TPAD - 1] = xp[0:F, :, 1:TPAD]           # t+1 shifted copy
        in_maps.append({"pack": pk})

    res = run_bass_kernel_spmd(nc, in_maps, core_ids=list(range(NCORES)))

    out = np.empty_like(x)
    for c in range(NCORES):
        flat0 = c * SEQ
        b0, n0 = divmod(flat0, N)
        out[b0, :, n0 : n0 + SEQ, :] = res.results[c]["yout"]
    return out

